# revision 35
# baseline (speedup 1.0000x reference)
"""Trainium2 Bass kernel for nn_Attention_27144193311525.

Computes, per (b, h):
    scores = exp(Q @ K^T) / sqrt(D)
    scores = where(mask == 0, -1e9, scores)
    p_attn = softmax(scores, axis=-1)
    out    = p_attn @ V
returns (out, p_attn) exactly like the reference.

Sharding: B*H = 16 head-pairs split across 8 NeuronCores (2 pairs/core).

Math notes (all equivalences are exact in fp32):
  - exp(z)/sqrt(64) == exp(z - ln 8): fold the scale into the ACT bias.
  - masking: zero the masked ROWS of K once per pair (k_masked = K * m01).
    Then z = q.k_masked = 0 at masked positions -> s = exp(0-ln8) = 0.125,
    which can never win the row max (unmasked s >= 0.125, and realistically
    max ~ e^18).  p = exp(s - max) underflows to exactly +0 at masked
    entries because max >> 88, matching the reference's -1e9 masking.
  - softmax denominator comes for free from the ACT accumulate port.
  - p is produced in fp16 (values in [0,1], ~5e-4 rel err): halves the
    dominant DRAM write (p_attn), feeds the PE transposes + PV matmul at
    1 cyc/row, and the host upcasts to fp32 on return.
  - QK^T runs fp32 (argmax-critical: the double-exp softmax is ~one-hot,
    so bf16-grade scores flip argmaxes and cost absmax ~1 errors); the
    64-deep contraction is 2-way row-packed into the 128x128 PE array.
"""

import math
import os
import sys

sys.path.insert(0, "/opt/trn_rl_repo")
os.environ.setdefault("MYCRO_LOCAL_CACHE", "1")

import numpy as np
from contextlib import ExitStack

import concourse.bass as bass
import concourse.tile as tile
from concourse import bacc, mybir
from concourse import bass_utils
from concourse.masks import make_identity

B, H, S, D = 2, 8, 2048, 64
N_CORES = 8
PAIRS_PER_CORE = (B * H) // N_CORES  # 2
P = 128
CHUNK = 512  # k-chunk for QK matmuls (one PSUM bank of fp32)
LN8 = math.log(8.0)

F32 = mybir.dt.float32
BF16 = mybir.dt.bfloat16
F16 = mybir.dt.float16
I32 = mybir.dt.int32
EXP = mybir.ActivationFunctionType.Exp
AXX = mybir.AxisListType.X

# 2-way row packing of the 64-deep QK contraction into the 128x128 PE array.
USE_PACK = True

# tunables (overridable before build_program for experiments)
CFG = dict(
    qk_chunk=512,     # psum chunk for QK matmuls / exp1 (512 or 1024)
    qk_bufs=4,        # psum bufs for qk chunks ([128,chunk] f32 = chunk/512 banks)
    tr_bufs=2,        # psum bufs for p-transpose tiles
    pv_bufs=2,        # psum bufs for PV accumulators
    s_bufs=4,         # sbuf bufs for s tiles
    p_bufs=2,         # sbuf bufs for p_h/pn/pT tiles
    ts_engine="vector",   # which engine does the p normalization tensor_scalar
    copy_split="vector",  # P^T psum->sbuf copies: "any", "vector", "split"
    tr_width=1024,        # width of transpose psum tiles (512/1024; f16 2KB=1 bank)
    vcast_engine="gpsimd",  # engine for the V fp32->fp16 cast (once per pair)
    # timing-knockout level (4=full, 3=no p_out DMA, 2=also no PV path,
    # 1=also no softmax stage (exp1 only)). Levels <4 give WRONG results.
    level=4,
)


def _kernel_body(ctx, tc, q_in, k_in, v_in, mask_in, p_out, o_out, n_pairs, s):
    nc = tc.nc
    nqt = s // P          # q tiles
    nkt = s // P          # k tiles
    chunk = min(CFG["qk_chunk"], s)
    nch = s // chunk      # k chunks for QK

    ts_eng = getattr(nc, CFG["ts_engine"])

    const = ctx.enter_context(tc.tile_pool(name="const", bufs=1))
    pair_pool = ctx.enter_context(tc.tile_pool(name="pair", bufs=2))
    work = ctx.enter_context(tc.tile_pool(name="work", bufs=CFG["p_bufs"]))
    stats = ctx.enter_context(tc.tile_pool(name="stats", bufs=4))
    psum_qk = ctx.enter_context(
        tc.tile_pool(name="psum_qk", bufs=CFG["qk_bufs"], space="PSUM")
    )
    psum_tr = ctx.enter_context(
        tc.tile_pool(name="psum_tr", bufs=CFG["tr_bufs"], space="PSUM")
    )
    psum_pv = ctx.enter_context(
        tc.tile_pool(name="psum_pv", bufs=CFG["pv_bufs"], space="PSUM")
    )

    ident_f = const.tile([P, P], F32)
    make_identity(nc, ident_f)
    ident_h = const.tile([P, P], F16)
    make_identity(nc, ident_h)

    # mask row -> m01[p, t] = mask[t*128 + p] as f32 (0.0 / 1.0)
    m01_i = const.tile([P, nkt], I32)
    nc.sync.dma_start(out=m01_i, in_=mask_in.rearrange("(t p) -> p t", p=P))
    m01 = const.tile([P, nkt], F32)
    nc.vector.tensor_copy(out=m01, in_=m01_i)

    # constant bias AP for exp(z - ln8)
    negln8 = const.tile([P, 1], F32)
    nc.vector.memset(negln8, -LN8)

    for pi in range(n_pairs):
        # ---- load Q, K, V in (t p) d layout: partition = row % 128 ----
        q_nat = pair_pool.tile([P, nqt, D], F32, bufs=2)
        nc.sync.dma_start(out=q_nat, in_=q_in[pi].rearrange("(t p) d -> p t d", p=P))
        k_nat = pair_pool.tile([P, nkt, D], F32, bufs=2)
        nc.sync.dma_start(out=k_nat, in_=k_in[pi].rearrange("(t p) d -> p t d", p=P))
        v_nat = pair_pool.tile([P, nkt, D], F32, bufs=2)
        nc.sync.dma_start(out=v_nat, in_=v_in[pi].rearrange("(t p) d -> p t d", p=P))

        v_h = pair_pool.tile([P, nkt, D], F16, bufs=2)
        getattr(nc, CFG["vcast_engine"]).tensor_copy(out=v_h, in_=v_nat)

        # zero masked K rows: masked keys then produce z=0 -> s=1/8 -> p=+0
        k_m = pair_pool.tile([P, nkt, D], F32, bufs=2)
        nc.vector.tensor_mul(k_m, k_nat, m01[:, :, None].broadcast_to([P, nkt, D]))

        # ---- build QT2/KT2 [128, s]: rows 0:64 = X^T, rows 64:128 = copy ----
        qt2 = pair_pool.tile([P, s], F32, bufs=2)
        kt2 = pair_pool.tile([P, s], F32, bufs=2)
        tiles_per_chunk = chunk // P
        for src, dst in ((q_nat, qt2), (k_m, kt2)):
            for g in range(nqt // tiles_per_chunk):
                setup_ps = psum_qk.tile([P, chunk], F32, tag="qkps", name="setup_ps")
                for j in range(tiles_per_chunk):
                    t = g * tiles_per_chunk + j
                    nc.tensor.transpose(
                        out=setup_ps[0:D, j * P : (j + 1) * P],
                        in_=src[:, t, :],
                        identity=ident_f,
                    )
                nc.any.tensor_copy(
                    out=dst[0:D, g * chunk : (g + 1) * chunk], in_=setup_ps[0:D, :]
                )
            if USE_PACK:
                # duplicate to partitions 64:128 for 2-way PE row packing
                nc.sync.dma_start(out=dst[D : 2 * D, :], in_=dst[0:D, :])

        out_sb = work.tile([P, nqt, D], F32, tag="out_sb")
        if CFG["level"] <= 2:
            nc.vector.memset(out_sb, 0.0)  # keep o_out DMA valid in knockouts

        # ---- per pair of q tiles ----
        for qj in range(nqt // 2):
            qa, qb = 2 * qj, 2 * qj + 1
            s_a = work.tile([P, s], F32, tag="s_sb", name="s_a", bufs=CFG["s_bufs"])
            s_b = work.tile([P, s], F32, tag="s_sb", name="s_b", bufs=CFG["s_bufs"])
            for c in range(nch):
                cs = slice(c * chunk, (c + 1) * chunk)
                ps_a = psum_qk.tile([P, chunk], F32, tag="qkps", name="ps_a")
                ps_b = psum_qk.tile([P, chunk], F32, tag="qkps", name="ps_b")
                for m0 in range(0, chunk, CHUNK):  # fp32 moving limit is 512
                    ms = slice(m0, m0 + CHUNK)
                    kcs = slice(c * chunk + m0, c * chunk + m0 + CHUNK)
                    if USE_PACK:
                        nc.tensor.matmul(
                            ps_a[:, ms], lhsT=qt2[0:D, qa * P : (qa + 1) * P],
                            rhs=kt2[0:D, kcs],
                            start=True, stop=True, tile_position=(0, 0),
                        )
                        nc.tensor.matmul(
                            ps_b[:, ms], lhsT=qt2[D : 2 * D, qb * P : (qb + 1) * P],
                            rhs=kt2[D : 2 * D, kcs],
                            start=True, stop=True, tile_position=(64, 0),
                        )
                    else:
                        nc.tensor.matmul(
                            ps_a[:, ms], lhsT=qt2[0:D, qa * P : (qa + 1) * P],
                            rhs=kt2[0:D, kcs],
                            start=True, stop=True,
                        )
                        nc.tensor.matmul(
                            ps_b[:, ms], lhsT=qt2[0:D, qb * P : (qb + 1) * P],
                            rhs=kt2[0:D, kcs],
                            start=True, stop=True,
                        )
                # s = exp(z - ln8) = exp(z)/sqrt(64)
                nc.scalar.activation(out=s_a[:, cs], in_=ps_a, func=EXP, bias=negln8)
                nc.scalar.activation(out=s_b[:, cs], in_=ps_b, func=EXP, bias=negln8)

            for qt_i, s_sb in ((qa, s_a), (qb, s_b)):
                if CFG["level"] <= 1:
                    continue  # exp1 only
                negmax = stats.tile([P, 1], F32, tag="negmax")
                nc.vector.reduce_max(negmax, s_sb, axis=AXX, negate=True)

                # p = exp(s - max) in fp16 (values in [0,1]; ~5e-4 rel err),
                # denominator accumulated in fp32 on the ACT side port.
                p_h = work.tile([P, s], F16, tag="p_h")
                den = stats.tile([P, 1], F32, tag="den")
                nc.scalar.activation(
                    out=p_h, in_=s_sb, func=EXP, bias=negmax, accum_out=den
                )
                rden = stats.tile([P, 1], F32, tag="rden")
                nc.vector.reciprocal(rden, den)

                # normalized p_attn (fp16) -> DRAM (host upcasts to fp32)
                pn = work.tile([P, s], F16, tag="pn", bufs=4)
                ts_eng.tensor_scalar_mul(pn, p_h, rden)
                if CFG["level"] >= 4:
                    dma_eng = nc.sync if qt_i % 2 == 0 else nc.scalar
                    dma_eng.dma_start(
                        out=p_out[pi, qt_i * P : (qt_i + 1) * P, :], in_=pn
                    )
                if CFG["level"] <= 2:
                    continue  # skip PV path

                # transpose p tiles: pT[:, k*128:(k+1)*128] = P^T for k-tile
                pT = work.tile([P, s], F16, tag="pT")
                trw = min(CFG["tr_width"], s)
                tpc = trw // P
                for g in range(nkt // tpc):
                    tr_ps = psum_tr.tile([P, trw], F16, tag="trps")
                    for j in range(tpc):
                        kk = g * tpc + j
                        nc.tensor.transpose(
                            out=tr_ps[:, j * P : (j + 1) * P],
                            in_=p_h[:, kk * P : (kk + 1) * P],
                            identity=ident_h,
                        )
                    dst_slice = pT[:, g * trw : (g + 1) * trw]
                    mode = CFG["copy_split"]
                    if mode == "any":
                        nc.any.tensor_copy(out=dst_slice, in_=tr_ps)
                    elif mode == "vector":
                        nc.vector.tensor_copy(out=dst_slice, in_=tr_ps)
                    elif mode == "split":
                        if g % 2 == 0:
                            nc.scalar.copy(out=dst_slice, in_=tr_ps)
                        else:
                            nc.vector.tensor_copy(out=dst_slice, in_=tr_ps)
                    else:
                        raise ValueError(mode)

                # out[q, :] = sum_k p^T[k, q] * V[k, :]
                pv = psum_pv.tile([P, D], F32, tag="pvps")
                for kt_i in range(nkt):
                    nc.tensor.matmul(
                        pv,
                        lhsT=pT[:, kt_i * P : (kt_i + 1) * P],
                        rhs=v_h[:, kt_i, :],
                        start=(kt_i == 0),
                        stop=(kt_i == nkt - 1),
                    )
                # fold 1/denominator into the small output tile
                nc.vector.tensor_scalar_mul(out_sb[:, qt_i, :], pv, rden)

        nc.sync.dma_start(
            out=o_out[pi].rearrange("(t p) d -> p t d", p=P), in_=out_sb
        )


def build_program(n_pairs=PAIRS_PER_CORE, s=S, reps=1):
    nc = bacc.Bacc(
        "TRN2", target_bir_lowering=False, debug=False, num_devices=N_CORES
    )
    q_in = nc.dram_tensor("q_in", [n_pairs, s, D], F32, kind="ExternalInput").ap()
    k_in = nc.dram_tensor("k_in", [n_pairs, s, D], F32, kind="ExternalInput").ap()
    v_in = nc.dram_tensor("v_in", [n_pairs, s, D], F32, kind="ExternalInput").ap()
    mask_in = nc.dram_tensor("mask_in", [s], I32, kind="ExternalInput").ap()
    p_out = nc.dram_tensor("p_out", [n_pairs, s, s], F16, kind="ExternalOutput").ap()
    o_out = nc.dram_tensor("o_out", [n_pairs, s, D], F32, kind="ExternalOutput").ap()

    with tile.TileContext(nc) as tc:
        for _ in range(reps):
            with ExitStack() as ctx:
                _kernel_body(
                    ctx, tc, q_in, k_in, v_in, mask_in, p_out, o_out, n_pairs, s
                )
    nc.compile()
    return nc


_CACHE = {}


def _get_program():
    if "nc" not in _CACHE:
        _CACHE["nc"] = build_program()
    return _CACHE["nc"]


def make_in_maps(query, key, value, mask):
    qf = np.ascontiguousarray(np.asarray(query, np.float32).reshape(B * H, S, D))
    kf = np.ascontiguousarray(np.asarray(key, np.float32).reshape(B * H, S, D))
    vf = np.ascontiguousarray(np.asarray(value, np.float32).reshape(B * H, S, D))
    mrow = np.ascontiguousarray(np.asarray(mask, np.int32).reshape(B, S))
    in_maps = []
    for c in range(N_CORES):
        lo = c * PAIRS_PER_CORE
        in_maps.append(
            {
                "q_in": qf[lo : lo + PAIRS_PER_CORE],
                "k_in": kf[lo : lo + PAIRS_PER_CORE],
                "v_in": vf[lo : lo + PAIRS_PER_CORE],
                "mask_in": mrow[lo // H],
            }
        )
    return in_maps


def assemble(results):
    p = (
        np.concatenate([r["p_out"] for r in results], axis=0)
        .reshape(B, H, S, S)
        .astype(np.float32)
    )
    o = np.concatenate([r["o_out"] for r in results], axis=0).reshape(B, H, S, D)
    return o, p


def kernel(query, key, value, mask, **run_kwargs):
    nc = _get_program()
    in_maps = make_in_maps(query, key, value, mask)
    res = bass_utils.run_bass_kernel_spmd(
        nc, in_maps, core_ids=list(range(N_CORES)), **run_kwargs
    )
    out = assemble(res.results)
    if run_kwargs:
        kernel.last_result = res  # stash for profiling harnesses
    return out


# revision 36
# speedup vs baseline: 1.9819x; 1.9819x over previous
"""Trainium2 Bass kernel for nn_Attention_27144193311525.

Computes, per (b, h):
    scores = exp(Q @ K^T) / sqrt(D)
    scores = where(mask == 0, -1e9, scores)
    p_attn = softmax(scores, axis=-1)
    out    = p_attn @ V
returns (out, p_attn) exactly like the reference.

Sharding: B*H = 16 head-pairs split across 8 NeuronCores (2 pairs/core).

Math notes (all equivalences are exact in fp32):
  - exp(z)/sqrt(64) == exp(z - ln 8): fold the scale into the ACT bias.
  - masking: zero the masked ROWS of K once per pair (k_masked = K * m01).
    Then z = q.k_masked = 0 at masked positions -> s = exp(0-ln8) = 0.125,
    which can never win the row max (unmasked s >= 0.125, and realistically
    max ~ e^18).  p = exp(s - max) underflows to exactly +0 at masked
    entries because max >> 88, matching the reference's -1e9 masking.
  - softmax denominator comes for free from the ACT accumulate port.
  - p is produced in fp16 (values in [0,1], ~5e-4 rel err): halves the
    dominant DRAM write (p_attn), feeds the PE transposes + PV matmul at
    1 cyc/row, and the host upcasts to fp32 on return.
  - QK^T runs fp32 (argmax-critical: the double-exp softmax is ~one-hot,
    so bf16-grade scores flip argmaxes and cost absmax ~1 errors); the
    64-deep contraction is 2-way row-packed into the 128x128 PE array.
"""

import math
import os
import sys

sys.path.insert(0, "/opt/trn_rl_repo")
os.environ.setdefault("MYCRO_LOCAL_CACHE", "1")

import numpy as np
from contextlib import ExitStack

import concourse.bass as bass
import concourse.tile as tile
from concourse import bacc, mybir
from concourse import bass_utils
from concourse.masks import make_identity

B, H, S, D = 2, 8, 2048, 64
N_CORES = 8
PAIRS_PER_CORE = (B * H) // N_CORES  # 2
P = 128
CHUNK = 512  # k-chunk for QK matmuls (one PSUM bank of fp32)
LN8 = math.log(8.0)

F32 = mybir.dt.float32
BF16 = mybir.dt.bfloat16
F16 = mybir.dt.float16
I32 = mybir.dt.int32
EXP = mybir.ActivationFunctionType.Exp
AXX = mybir.AxisListType.X

# 2-way row packing of the 64-deep QK contraction into the 128x128 PE array.
USE_PACK = True

# tunables (overridable before build_program for experiments)
CFG = dict(
    qk_chunk=1024,    # psum chunk for QK matmuls / exp1 (512 or 1024)
    qk_bufs=3,        # psum bufs for qk chunks ([128,chunk] f32 = chunk/512 banks)
    tr_bufs=1,        # psum bufs for p-transpose tiles
    pv_bufs=1,        # psum bufs for PV accumulators
    s_bufs=4,         # sbuf bufs for s tiles
    p_bufs=2,         # sbuf bufs for p_h/pn/pT tiles
    ts_engine="vector",   # which engine does the p normalization tensor_scalar
    copy_split="vector",  # P^T psum->sbuf copies: "any", "vector", "split"
    tr_width=1024,        # width of transpose psum tiles (512/1024; f16 2KB=1 bank)
    vcast_engine="gpsimd",  # engine for the V fp32->fp16 cast (once per pair)
    # timing-knockout level (4=full, 3=no p_out DMA, 2=also no PV path,
    # 1=also no softmax stage (exp1 only)). Levels <4 give WRONG results.
    level=4,
)


def _kernel_body(ctx, tc, q_in, k_in, v_in, mask_in, p_out, o_out, n_pairs, s):
    nc = tc.nc
    nqt = s // P          # q tiles
    nkt = s // P          # k tiles
    chunk = min(CFG["qk_chunk"], s)
    nch = s // chunk      # k chunks for QK

    ts_eng = getattr(nc, CFG["ts_engine"])

    const = ctx.enter_context(tc.tile_pool(name="const", bufs=1))
    pair_pool = ctx.enter_context(tc.tile_pool(name="pair", bufs=2))
    work = ctx.enter_context(tc.tile_pool(name="work", bufs=CFG["p_bufs"]))
    stats = ctx.enter_context(tc.tile_pool(name="stats", bufs=4))
    psum_qk = ctx.enter_context(
        tc.tile_pool(name="psum_qk", bufs=CFG["qk_bufs"], space="PSUM")
    )
    psum_tr = ctx.enter_context(
        tc.tile_pool(name="psum_tr", bufs=CFG["tr_bufs"], space="PSUM")
    )
    psum_pv = ctx.enter_context(
        tc.tile_pool(name="psum_pv", bufs=CFG["pv_bufs"], space="PSUM")
    )

    ident_f = const.tile([P, P], F32)
    make_identity(nc, ident_f)
    ident_h = const.tile([P, P], F16)
    make_identity(nc, ident_h)

    # mask row -> m01[p, t] = mask[t*128 + p] as f32 (0.0 / 1.0)
    m01_i = const.tile([P, nkt], I32)
    nc.sync.dma_start(out=m01_i, in_=mask_in.rearrange("(t p) -> p t", p=P))
    m01 = const.tile([P, nkt], F32)
    nc.vector.tensor_copy(out=m01, in_=m01_i)

    # constant bias AP for exp(z - ln8)
    negln8 = const.tile([P, 1], F32)
    nc.vector.memset(negln8, -LN8)

    for pi in range(n_pairs):
        # ---- load Q, K, V in (t p) d layout: partition = row % 128 ----
        q_nat = pair_pool.tile([P, nqt, D], F32, bufs=2)
        nc.sync.dma_start(out=q_nat, in_=q_in[pi].rearrange("(t p) d -> p t d", p=P))
        k_nat = pair_pool.tile([P, nkt, D], F32, bufs=2)
        nc.sync.dma_start(out=k_nat, in_=k_in[pi].rearrange("(t p) d -> p t d", p=P))
        v_nat = pair_pool.tile([P, nkt, D], F32, bufs=2)
        nc.sync.dma_start(out=v_nat, in_=v_in[pi].rearrange("(t p) d -> p t d", p=P))

        v_h = pair_pool.tile([P, nkt, D], F16, bufs=2)
        getattr(nc, CFG["vcast_engine"]).tensor_copy(out=v_h, in_=v_nat)

        # zero masked K rows: masked keys then produce z=0 -> s=1/8 -> p=+0
        k_m = pair_pool.tile([P, nkt, D], F32, bufs=2)
        nc.vector.tensor_mul(k_m, k_nat, m01[:, :, None].broadcast_to([P, nkt, D]))

        # ---- build QT2/KT2 [128, s]: rows 0:64 = X^T, rows 64:128 = copy ----
        qt2 = pair_pool.tile([P, s], F32, bufs=2)
        kt2 = pair_pool.tile([P, s], F32, bufs=2)
        tiles_per_chunk = chunk // P
        for src, dst in ((q_nat, qt2), (k_m, kt2)):
            for g in range(nqt // tiles_per_chunk):
                setup_ps = psum_qk.tile([P, chunk], F32, tag="qkps", name="setup_ps")
                for j in range(tiles_per_chunk):
                    t = g * tiles_per_chunk + j
                    nc.tensor.transpose(
                        out=setup_ps[0:D, j * P : (j + 1) * P],
                        in_=src[:, t, :],
                        identity=ident_f,
                    )
                nc.any.tensor_copy(
                    out=dst[0:D, g * chunk : (g + 1) * chunk], in_=setup_ps[0:D, :]
                )
            if USE_PACK:
                # duplicate to partitions 64:128 for 2-way PE row packing
                nc.sync.dma_start(out=dst[D : 2 * D, :], in_=dst[0:D, :])

        out_sb = work.tile([P, nqt, D], F32, tag="out_sb")
        if CFG["level"] <= 2:
            nc.vector.memset(out_sb, 0.0)  # keep o_out DMA valid in knockouts

        # ---- per pair of q tiles ----
        for qj in range(nqt // 2):
            qa, qb = 2 * qj, 2 * qj + 1
            s_a = work.tile([P, s], F32, tag="s_sb", name="s_a", bufs=CFG["s_bufs"])
            s_b = work.tile([P, s], F32, tag="s_sb", name="s_b", bufs=CFG["s_bufs"])
            for c in range(nch):
                cs = slice(c * chunk, (c + 1) * chunk)
                ps_a = psum_qk.tile([P, chunk], F32, tag="qkps", name="ps_a")
                ps_b = psum_qk.tile([P, chunk], F32, tag="qkps", name="ps_b")
                for m0 in range(0, chunk, CHUNK):  # fp32 moving limit is 512
                    ms = slice(m0, m0 + CHUNK)
                    kcs = slice(c * chunk + m0, c * chunk + m0 + CHUNK)
                    if USE_PACK:
                        nc.tensor.matmul(
                            ps_a[:, ms], lhsT=qt2[0:D, qa * P : (qa + 1) * P],
                            rhs=kt2[0:D, kcs],
                            start=True, stop=True, tile_position=(0, 0),
                        )
                        nc.tensor.matmul(
                            ps_b[:, ms], lhsT=qt2[D : 2 * D, qb * P : (qb + 1) * P],
                            rhs=kt2[D : 2 * D, kcs],
                            start=True, stop=True, tile_position=(64, 0),
                        )
                    else:
                        nc.tensor.matmul(
                            ps_a[:, ms], lhsT=qt2[0:D, qa * P : (qa + 1) * P],
                            rhs=kt2[0:D, kcs],
                            start=True, stop=True,
                        )
                        nc.tensor.matmul(
                            ps_b[:, ms], lhsT=qt2[0:D, qb * P : (qb + 1) * P],
                            rhs=kt2[0:D, kcs],
                            start=True, stop=True,
                        )
                # s = exp(z - ln8) = exp(z)/sqrt(64)
                nc.scalar.activation(out=s_a[:, cs], in_=ps_a, func=EXP, bias=negln8)
                nc.scalar.activation(out=s_b[:, cs], in_=ps_b, func=EXP, bias=negln8)

            for qt_i, s_sb in ((qa, s_a), (qb, s_b)):
                if CFG["level"] <= 1:
                    continue  # exp1 only
                negmax = stats.tile([P, 1], F32, tag="negmax")
                nc.vector.reduce_max(negmax, s_sb, axis=AXX, negate=True)

                # p = exp(s - max) in fp16 (values in [0,1]; ~5e-4 rel err),
                # denominator accumulated in fp32 on the ACT side port.
                p_h = work.tile([P, s], F16, tag="p_h")
                den = stats.tile([P, 1], F32, tag="den")
                nc.scalar.activation(
                    out=p_h, in_=s_sb, func=EXP, bias=negmax, accum_out=den
                )
                rden = stats.tile([P, 1], F32, tag="rden")
                nc.vector.reciprocal(rden, den)

                # normalized p_attn (fp16) -> DRAM (host upcasts to fp32)
                pn = work.tile([P, s], F16, tag="pn", bufs=4)
                ts_eng.tensor_scalar_mul(pn, p_h, rden)
                if CFG["level"] >= 4:
                    dma_eng = nc.sync if qt_i % 2 == 0 else nc.scalar
                    dma_eng.dma_start(
                        out=p_out[pi, qt_i * P : (qt_i + 1) * P, :], in_=pn
                    )
                if CFG["level"] <= 2:
                    continue  # skip PV path

                # transpose p tiles: pT[:, k*128:(k+1)*128] = P^T for k-tile
                pT = work.tile([P, s], F16, tag="pT")
                trw = min(CFG["tr_width"], s)
                tpc = trw // P
                for g in range(nkt // tpc):
                    tr_ps = psum_tr.tile([P, trw], F16, tag="trps")
                    for j in range(tpc):
                        kk = g * tpc + j
                        nc.tensor.transpose(
                            out=tr_ps[:, j * P : (j + 1) * P],
                            in_=p_h[:, kk * P : (kk + 1) * P],
                            identity=ident_h,
                        )
                    dst_slice = pT[:, g * trw : (g + 1) * trw]
                    mode = CFG["copy_split"]
                    if mode == "any":
                        nc.any.tensor_copy(out=dst_slice, in_=tr_ps)
                    elif mode == "vector":
                        nc.vector.tensor_copy(out=dst_slice, in_=tr_ps)
                    elif mode == "split":
                        if g % 2 == 0:
                            nc.scalar.copy(out=dst_slice, in_=tr_ps)
                        else:
                            nc.vector.tensor_copy(out=dst_slice, in_=tr_ps)
                    else:
                        raise ValueError(mode)

                # out[q, :] = sum_k p^T[k, q] * V[k, :]
                pv = psum_pv.tile([P, D], F32, tag="pvps")
                for kt_i in range(nkt):
                    nc.tensor.matmul(
                        pv,
                        lhsT=pT[:, kt_i * P : (kt_i + 1) * P],
                        rhs=v_h[:, kt_i, :],
                        start=(kt_i == 0),
                        stop=(kt_i == nkt - 1),
                    )
                # fold 1/denominator into the small output tile
                nc.vector.tensor_scalar_mul(out_sb[:, qt_i, :], pv, rden)

        nc.sync.dma_start(
            out=o_out[pi].rearrange("(t p) d -> p t d", p=P), in_=out_sb
        )


def build_program(n_pairs=PAIRS_PER_CORE, s=S, reps=1):
    nc = bacc.Bacc(
        "TRN2", target_bir_lowering=False, debug=False, num_devices=N_CORES
    )
    q_in = nc.dram_tensor("q_in", [n_pairs, s, D], F32, kind="ExternalInput").ap()
    k_in = nc.dram_tensor("k_in", [n_pairs, s, D], F32, kind="ExternalInput").ap()
    v_in = nc.dram_tensor("v_in", [n_pairs, s, D], F32, kind="ExternalInput").ap()
    mask_in = nc.dram_tensor("mask_in", [s], I32, kind="ExternalInput").ap()
    p_out = nc.dram_tensor("p_out", [n_pairs, s, s], F16, kind="ExternalOutput").ap()
    o_out = nc.dram_tensor("o_out", [n_pairs, s, D], F32, kind="ExternalOutput").ap()

    with tile.TileContext(nc) as tc:
        for _ in range(reps):
            with ExitStack() as ctx:
                _kernel_body(
                    ctx, tc, q_in, k_in, v_in, mask_in, p_out, o_out, n_pairs, s
                )
    nc.compile()
    return nc


_CACHE = {}


def _get_program():
    if "nc" not in _CACHE:
        _CACHE["nc"] = build_program()
    return _CACHE["nc"]


def make_in_maps(query, key, value, mask):
    qf = np.ascontiguousarray(np.asarray(query, np.float32).reshape(B * H, S, D))
    kf = np.ascontiguousarray(np.asarray(key, np.float32).reshape(B * H, S, D))
    vf = np.ascontiguousarray(np.asarray(value, np.float32).reshape(B * H, S, D))
    mrow = np.ascontiguousarray(np.asarray(mask, np.int32).reshape(B, S))
    in_maps = []
    for c in range(N_CORES):
        lo = c * PAIRS_PER_CORE
        in_maps.append(
            {
                "q_in": qf[lo : lo + PAIRS_PER_CORE],
                "k_in": kf[lo : lo + PAIRS_PER_CORE],
                "v_in": vf[lo : lo + PAIRS_PER_CORE],
                "mask_in": mrow[lo // H],
            }
        )
    return in_maps


def assemble(results):
    p = (
        np.concatenate([r["p_out"] for r in results], axis=0)
        .reshape(B, H, S, S)
        .astype(np.float32)
    )
    o = np.concatenate([r["o_out"] for r in results], axis=0).reshape(B, H, S, D)
    return o, p


def kernel(query, key, value, mask, **run_kwargs):
    nc = _get_program()
    in_maps = make_in_maps(query, key, value, mask)
    res = bass_utils.run_bass_kernel_spmd(
        nc, in_maps, core_ids=list(range(N_CORES)), **run_kwargs
    )
    out = assemble(res.results)
    if run_kwargs:
        kernel.last_result = res  # stash for profiling harnesses
    return out
